# Initial kernel scaffold
#
"""Bass/Trainium2 kernel for nn_ChannelAttention (sparse_attention).

Math: per (batch b, 32-channel block n), q/k/v are per-channel affine maps of
x rows: q_d = A_d*x_d + B_d etc.  Hence q.k^T, the l2 norms, and attn@v are all
functions of the per-block channel Gram matrix G = X X^T and row sums S = X@1.
The whole module collapses to out[b] = BlockDiag(M_n) @ x[b] + beta, where the
M_n are 32x32 matrices derived from G,S via 16 tiny softmaxes (done on host).

Phase 1 (device, sharded over pixels): per-core partial [G | S].  x is cast to
  bf16 in-flight (SWDGE cast DMA), transposed on-chip via x-bar DMA transpose,
  and accumulated as 1-pass bf16 matmuls [G|S] += xt.T @ [xt | 1] in PSUM.
  (bf16 stats perturb the final output by ~1e-5 relative - the logits live in
  [-1,1] and divide by norms ~|A|*sqrt(N), so absolute Gram error ~1 vanishes.)
Host: reduce partials, tiny softmax math -> M^T (block-diagonal 128x128), beta.
Phase 2 (device, sharded over pixels): out = M @ x + beta, fp32.
"""

import numpy as np

import concourse.bacc as bacc
import concourse.mybir as mybir
import concourse.tile as tile
import concourse.bass_utils as bass_utils
from concourse.masks import make_identity

B, C, H, W = 2, 128, 256, 256
HW = H * W
NCORES = 8
SH = HW // NCORES  # 8192 pixels per core
E = 2
NCH = 4
HEADS = NCH * E
D = C // NCH  # 32
EPS = 1e-12
F32 = mybir.dt.float32
BF16 = mybir.dt.bfloat16
FP16 = mybir.dt.float16

CH = 2048  # dma chunk columns

_cache = {}


def _build_phase1(mode="pet"):
    nc = bacc.Bacc("TRN2", target_bir_lowering=False, debug=False, num_devices=NCORES)
    x = nc.dram_tensor("x", [B, C, SH], F32, kind="ExternalInput").ap()
    idd = nc.dram_tensor("idd", [C, C], BF16, kind="ExternalInput").ap()
    gs = nc.dram_tensor("gs", [B, C, 129], F32, kind="ExternalOutput").ap()
    GRP = 4  # transposed chunks per PSUM bank / ACT copy group
    with tile.TileContext(nc) as tc:
        with (
            tc.tile_pool(name="const", bufs=1) as constp,
            tc.tile_pool(name="xin", bufs=4) as xinp,
            tc.tile_pool(name="xt", bufs=4) as xtp,
            tc.tile_pool(name="xtps", bufs=4, space="PSUM") as xtpsp,
            tc.tile_pool(name="gram", bufs=2, space="PSUM") as gramp,
            tc.tile_pool(name="gout", bufs=2) as goutp,
        ):
            ident = constp.tile([128, 128], BF16)
            nc.sync.dma_start(out=ident, in_=idd)
            nchunks = SH // 128  # 64 per batch
            ngrp = nchunks // GRP
            pending = None  # software pipeline: grams lag one group
            gabs = 0  # global group counter (xt pool slot = gabs % bufs)

            def emit_grams(args):
                gram_t, xt_sb, j0 = args
                for i in range(GRP):
                    j = j0 + i
                    nc.tensor.matmul(gram_t[:, 0:129],
                                     lhsT=xt_sb[:, i, 0:128],
                                     rhs=xt_sb[:, i, 0:129],
                                     start=(j == 0), stop=(j == nchunks - 1))

            for b in range(B):
                gram = gramp.tile([128, 132], F32, tag="gram")
                for g in range(ngrp):  # 16 groups of 4 chunks
                    jc, kg = divmod(g, (CH // 128) // GRP)  # 4 groups per dma chunk
                    if kg == 0:
                        xb = xinp.tile([128, CH], BF16, tag="xin")
                        # SWDGE cast dma: fp32 HBM -> bf16 SBUF, split into
                        # sub-transfers so the first transposes start early
                        for s in range(4):
                            w0 = s * (CH // 4)
                            nc.gpsimd.dma_start(
                                out=xb[:, w0:w0 + CH // 4],
                                in_=x[b, :, jc * CH + w0:jc * CH + w0 + CH // 4])
                    xt_ps = xtpsp.tile([128, GRP * 128], F32, tag="xtps")
                    for i in range(GRP):
                        k = kg * GRP + i
                        nc.tensor.matmul(xt_ps[:, i * 128:(i + 1) * 128],
                                         lhsT=xb[:, k * 128:(k + 1) * 128],
                                         rhs=ident, start=True, stop=True)
                    if pending is not None:
                        emit_grams(pending)
                    xt_sb = xtp.tile([128, GRP, 132], BF16, tag="xt")
                    # alternate copyback between ACT and DVE so neither paces
                    if g % 2 == 0:
                        nc.scalar.copy(
                            xt_sb[:, :, 0:128],
                            xt_ps.rearrange("p (g f) -> p g f", g=GRP))
                    else:
                        nc.vector.tensor_copy(
                            xt_sb[:, :, 0:128],
                            xt_ps.rearrange("p (g f) -> p g f", g=GRP))
                    if gabs < 4:  # ones col survives copybacks; set once per slot
                        nc.vector.memset(xt_sb[:, :, 128:129], 1.0)
                    gabs += 1
                    pending = (gram, xt_sb, g * GRP)
                emit_grams(pending)
                pending = None
                go = goutp.tile([128, 129], F32, tag="gout")
                nc.vector.tensor_copy(go, gram[:, 0:129])
                nc.sync.dma_start(out=gs[b], in_=go)
    nc.compile()
    return nc


def _build_phase2():
    nc = bacc.Bacc("TRN2", target_bir_lowering=False, debug=False, num_devices=NCORES)
    x = nc.dram_tensor("x", [B, C, SH], F32, kind="ExternalInput").ap()
    mt = nc.dram_tensor("mt", [B, C, C], F32, kind="ExternalInput").ap()
    beta = nc.dram_tensor("beta", [B, C, 1], F32, kind="ExternalInput").ap()
    out = nc.dram_tensor("out", [B, C, SH], F32, kind="ExternalOutput").ap()
    with tile.TileContext(nc) as tc:
        with (
            tc.tile_pool(name="wts", bufs=1) as wp,
            tc.tile_pool(name="xin", bufs=6) as xinp,
            tc.tile_pool(name="ps", bufs=8, space="PSUM") as psp,
            tc.tile_pool(name="osb", bufs=6) as osbp,
        ):
            mts, betas = [], []
            for b in range(B):
                mt_sb = wp.tile([128, 128], F32, tag=f"mt{b}")
                nc.scalar.dma_start(out=mt_sb, in_=mt[b])
                beta_sb = wp.tile([128, 1], F32, tag=f"beta{b}")
                nc.scalar.dma_start(out=beta_sb, in_=beta[b])
                mts.append(mt_sb)
                betas.append(beta_sb)
            for b in range(B):
                mt_sb, beta_sb = mts[b], betas[b]
                for jc in range(SH // CH):  # 4
                    x_t = xinp.tile([128, CH], F32, tag="xin")
                    # only the very first chunk needs fine fill granularity;
                    # later chunks as whole transfers cut SP descriptor-gen work
                    if b == 0 and jc == 0:
                        splits = (512, 512, 1024)
                    else:
                        splits = (CH,)
                    w0 = 0
                    for w in splits:
                        nc.sync.dma_start(
                            out=x_t[:, w0:w0 + w],
                            in_=x[b, :, jc * CH + w0:jc * CH + w0 + w])
                        w0 += w
                    o_sb = osbp.tile([128, CH], F32, tag="osb")
                    for k in range(CH // 512):  # 4
                        ps = psp.tile([128, 512], F32, tag="ps")
                        nc.tensor.matmul(ps, lhsT=mt_sb,
                                         rhs=x_t[:, k * 512:(k + 1) * 512],
                                         start=True, stop=True)
                        dst = o_sb[:, k * 512:(k + 1) * 512]
                        if k % 2 == 0:
                            nc.vector.tensor_scalar_add(dst, in0=ps,
                                                        scalar1=beta_sb)
                        else:
                            nc.scalar.add(dst, ps, beta_sb)
                        nc.scalar.dma_start(
                            out=out[b, :, jc * CH + k * 512:jc * CH + (k + 1) * 512],
                            in_=dst)
    nc.compile()
    return nc


def _softmax(a, axis=-1):
    m = np.max(a, axis=axis, keepdims=True)
    ex = np.exp(a - m)
    return ex / np.sum(ex, axis=axis, keepdims=True)


def _host_mbeta(G, S, w_qkv, b_qkv, w_fus, b_fus, t):
    """From per-batch Gram G [B,128,128] and row sums S [B,128], build
    M^T [B,128,128] (block-diagonal) and beta [B,128,1]."""
    N = float(HW)
    t = t.reshape(HEADS)
    M = np.zeros((B, C, C), dtype=np.float64)
    beta = np.zeros((B, C), dtype=np.float64)
    for b in range(B):
        for n in range(NCH):
            sl = slice(n * D, (n + 1) * D)
            Gb = G[b][sl, sl]
            dG = np.diag(Gb)
            Sb = S[b][sl]
            Mn = np.zeros((D, D), dtype=np.float64)
            bn = np.zeros(D, dtype=np.float64)
            for e in range(E):
                h = e * NCH + n
                A = w_qkv[sl, e]
                Bv = b_qkv[sl, e]
                Cv = w_qkv[sl, E + e]
                Dv = b_qkv[sl, E + e]
                Vv = w_qkv[sl, 2 * E + e]
                Uv = b_qkv[sl, 2 * E + e]
                qk = ((A[:, None] * Cv[None, :]) * Gb
                      + (A * Sb)[:, None] * Dv[None, :]
                      + Bv[:, None] * (Cv * Sb)[None, :]
                      + N * (Bv[:, None] * Dv[None, :]))
                nq = np.sqrt(np.maximum(A * A * dG + 2 * A * Bv * Sb + Bv * Bv * N, 0.0))
                nk = np.sqrt(np.maximum(Cv * Cv * dG + 2 * Cv * Dv * Sb + Dv * Dv * N, 0.0))
                L = t[h] * qk / np.maximum(nq, EPS)[:, None] / np.maximum(nk, EPS)[None, :]
                P = _softmax(L, axis=-1)
                Mn += w_fus[sl, e][:, None] * (P * Vv[None, :])
                bn += w_fus[sl, e] * (P @ Uv)
            bn += b_fus[sl]
            M[b][sl, sl] = Mn
            beta[b][sl] = bn
    mtr = np.ascontiguousarray(M.transpose(0, 2, 1)).astype(np.float32)
    return mtr, beta.astype(np.float32).reshape(B, C, 1)


def kernel(x, w_qkv, b_qkv, w_fus, b_fus, t, _profile=None):
    x = np.asarray(x, dtype=np.float32)
    w_qkv = np.asarray(w_qkv, dtype=np.float64)
    b_qkv = np.asarray(b_qkv, dtype=np.float64)
    w_fus = np.asarray(w_fus, dtype=np.float64)
    b_fus = np.asarray(b_fus, dtype=np.float64)
    t = np.asarray(t, dtype=np.float64)

    mode = (_profile or {}).get("p1mode", "pet")
    key1 = f"p1-{mode}"
    if key1 not in _cache:
        _cache[key1] = _build_phase1(mode)
    if "p2" not in _cache:
        _cache["p2"] = _build_phase2()

    import ml_dtypes
    xf = x.reshape(B, C, HW)
    shards = [np.ascontiguousarray(xf[:, :, i * SH:(i + 1) * SH])
              for i in range(NCORES)]

    kw = {}
    if _profile and _profile.get("trace"):
        kw["trace"] = True
    idd = np.eye(C, dtype=ml_dtypes.bfloat16)
    res1 = bass_utils.run_bass_kernel_spmd(
        _cache[key1], [{"x": s, "idd": idd} for s in shards],
        core_ids=list(range(NCORES)), **kw)
    gs = np.sum([r["gs"].astype(np.float64) for r in res1.results], axis=0)
    G = gs[:, :, 0:128]
    S = gs[:, :, 128]

    mtr, beta = _host_mbeta(G, S, w_qkv, b_qkv, w_fus, b_fus, t)
    res2 = bass_utils.run_bass_kernel_spmd(
        _cache["p2"],
        [{"x": s, "mt": mtr, "beta": beta} for s in shards],
        core_ids=list(range(NCORES)), **kw)
    out = np.concatenate([r["out"] for r in res2.results], axis=2)
    if _profile is not None:
        _profile["results"] = (res1, res2)
    return out.reshape(B, C, H, W)



# revision 1
# speedup vs baseline: 1.5417x; 1.5417x over previous
"""Bass/Trainium2 kernel for nn_ChannelAttention (sparse_attention).

Math: per (batch b, 32-channel block n), q/k/v are per-channel affine maps of
x rows: q_d = A_d*x_d + B_d etc.  Hence q.k^T, the l2 norms, and attn@v are all
functions of the per-block channel Gram matrix G = X X^T and row sums S = X@1.
The whole module collapses to out[b] = BlockDiag(M_n) @ x[b] + beta, where the
M_n are 32x32 matrices derived from G,S via 16 tiny softmaxes (done on host).

Phase 1 (device, sharded over pixels): per-core partial [G | S].  x is cast to
  bf16 in-flight (SWDGE cast DMA), transposed on-chip via x-bar DMA transpose,
  and accumulated as 1-pass bf16 matmuls [G|S] += xt.T @ [xt | 1] in PSUM.
  (bf16 stats perturb the final output by ~1e-5 relative - the logits live in
  [-1,1] and divide by norms ~|A|*sqrt(N), so absolute Gram error ~1 vanishes.)
Host: reduce partials, tiny softmax math -> M^T (block-diagonal 128x128), beta.
Phase 2 (device, sharded over pixels): out = M @ x + beta, fp32.
"""

import numpy as np

import concourse.bacc as bacc
import concourse.mybir as mybir
import concourse.tile as tile
import concourse.bass_utils as bass_utils
from concourse.masks import make_identity

B, C, H, W = 2, 128, 256, 256
HW = H * W
NCORES = 8
SH = HW // NCORES  # 8192 pixels per core
E = 2
NCH = 4
HEADS = NCH * E
D = C // NCH  # 32
EPS = 1e-12
F32 = mybir.dt.float32
BF16 = mybir.dt.bfloat16
FP16 = mybir.dt.float16

CH = 2048  # dma chunk columns

_cache = {}


def _build_phase1(mode="pet"):
    nc = bacc.Bacc("TRN2", target_bir_lowering=False, debug=False, num_devices=NCORES)
    x = nc.dram_tensor("x", [B, C, SH], F32, kind="ExternalInput").ap()
    idd = nc.dram_tensor("idd", [C, C], BF16, kind="ExternalInput").ap()
    gs = nc.dram_tensor("gs", [B, C, 129], F32, kind="ExternalOutput").ap()
    GRP = 4  # transposed chunks per PSUM bank / ACT copy group
    with tile.TileContext(nc) as tc:
        with (
            tc.tile_pool(name="const", bufs=1) as constp,
            tc.tile_pool(name="xin", bufs=4) as xinp,
            tc.tile_pool(name="xt", bufs=4) as xtp,
            tc.tile_pool(name="xtps", bufs=4, space="PSUM") as xtpsp,
            tc.tile_pool(name="gram", bufs=2, space="PSUM") as gramp,
            tc.tile_pool(name="gout", bufs=2) as goutp,
        ):
            ident = constp.tile([128, 128], BF16)
            nc.sync.dma_start(out=ident, in_=idd)
            nchunks = SH // 128  # 64 per batch
            ngrp = nchunks // GRP
            pending = None  # software pipeline: grams lag one group
            gabs = 0  # global group counter (xt pool slot = gabs % bufs)

            def emit_grams(args):
                gram_t, xt_sb, j0 = args
                for i in range(GRP):
                    j = j0 + i
                    nc.tensor.matmul(gram_t[:, 0:129],
                                     lhsT=xt_sb[:, i, 0:128],
                                     rhs=xt_sb[:, i, 0:129],
                                     start=(j == 0), stop=(j == nchunks - 1))

            for b in range(B):
                gram = gramp.tile([128, 132], F32, tag="gram")
                for g in range(ngrp):  # 16 groups of 4 chunks
                    jc, kg = divmod(g, (CH // 128) // GRP)  # 4 groups per dma chunk
                    if kg == 0:
                        xb = xinp.tile([128, CH], BF16, tag="xin")
                        # SWDGE cast dma: fp32 HBM -> bf16 SBUF, split into
                        # sub-transfers so the first transposes start early
                        for s in range(4):
                            w0 = s * (CH // 4)
                            nc.gpsimd.dma_start(
                                out=xb[:, w0:w0 + CH // 4],
                                in_=x[b, :, jc * CH + w0:jc * CH + w0 + CH // 4])
                    xt_ps = xtpsp.tile([128, GRP * 128], F32, tag="xtps")
                    for i in range(GRP):
                        k = kg * GRP + i
                        nc.tensor.matmul(xt_ps[:, i * 128:(i + 1) * 128],
                                         lhsT=xb[:, k * 128:(k + 1) * 128],
                                         rhs=ident, start=True, stop=True)
                    if pending is not None:
                        emit_grams(pending)
                    xt_sb = xtp.tile([128, GRP, 132], BF16, tag="xt")
                    # alternate copyback between ACT and DVE so neither paces
                    if g % 2 == 0:
                        nc.scalar.copy(
                            xt_sb[:, :, 0:128],
                            xt_ps.rearrange("p (g f) -> p g f", g=GRP))
                    else:
                        nc.vector.tensor_copy(
                            xt_sb[:, :, 0:128],
                            xt_ps.rearrange("p (g f) -> p g f", g=GRP))
                    if gabs < 4:  # ones col survives copybacks; set once per slot
                        nc.vector.memset(xt_sb[:, :, 128:129], 1.0)
                    gabs += 1
                    pending = (gram, xt_sb, g * GRP)
                emit_grams(pending)
                pending = None
                go = goutp.tile([128, 129], F32, tag="gout")
                nc.vector.tensor_copy(go, gram[:, 0:129])
                nc.sync.dma_start(out=gs[b], in_=go)
    nc.compile()
    return nc


def _build_phase2():
    nc = bacc.Bacc("TRN2", target_bir_lowering=False, debug=False, num_devices=NCORES)
    x = nc.dram_tensor("x", [B, C, SH], F32, kind="ExternalInput").ap()
    mt = nc.dram_tensor("mt", [B, C, C], F32, kind="ExternalInput").ap()
    beta = nc.dram_tensor("beta", [B, C, 1], F32, kind="ExternalInput").ap()
    out = nc.dram_tensor("out", [B, C, SH], F32, kind="ExternalOutput").ap()
    with tile.TileContext(nc) as tc:
        with (
            tc.tile_pool(name="wts", bufs=1) as wp,
            tc.tile_pool(name="xin", bufs=6) as xinp,
            tc.tile_pool(name="ps", bufs=8, space="PSUM") as psp,
            tc.tile_pool(name="osb", bufs=6) as osbp,
        ):
            mts, betas = [], []
            for b in range(B):
                mt_sb = wp.tile([128, 128], F32, tag=f"mt{b}")
                nc.scalar.dma_start(out=mt_sb, in_=mt[b])
                beta_sb = wp.tile([128, 1], F32, tag=f"beta{b}")
                nc.scalar.dma_start(out=beta_sb, in_=beta[b])
                mts.append(mt_sb)
                betas.append(beta_sb)
            for b in range(B):
                mt_sb, beta_sb = mts[b], betas[b]
                for jc in range(SH // CH):  # 4
                    x_t = xinp.tile([128, CH], F32, tag="xin")
                    # only the very first chunk needs fine fill granularity;
                    # later chunks as whole transfers cut SP descriptor-gen work
                    if b == 0 and jc == 0:
                        splits = (512, 512, 1024)
                    else:
                        splits = (CH,)
                    w0 = 0
                    for w in splits:
                        nc.sync.dma_start(
                            out=x_t[:, w0:w0 + w],
                            in_=x[b, :, jc * CH + w0:jc * CH + w0 + w])
                        w0 += w
                    o_sb = osbp.tile([128, CH], F32, tag="osb")
                    for k in range(CH // 512):  # 4
                        ps = psp.tile([128, 512], F32, tag="ps")
                        nc.tensor.matmul(ps, lhsT=mt_sb,
                                         rhs=x_t[:, k * 512:(k + 1) * 512],
                                         start=True, stop=True)
                        dst = o_sb[:, k * 512:(k + 1) * 512]
                        if k % 2 == 0:
                            nc.vector.tensor_scalar_add(dst, in0=ps,
                                                        scalar1=beta_sb)
                        else:
                            nc.scalar.add(dst, ps, beta_sb)
                        nc.scalar.dma_start(
                            out=out[b, :, jc * CH + k * 512:jc * CH + (k + 1) * 512],
                            in_=dst)
    nc.compile()
    return nc


def _softmax(a, axis=-1):
    m = np.max(a, axis=axis, keepdims=True)
    ex = np.exp(a - m)
    return ex / np.sum(ex, axis=axis, keepdims=True)


def _host_mbeta(G, S, w_qkv, b_qkv, w_fus, b_fus, t):
    """From per-batch Gram G [B,128,128] and row sums S [B,128], build
    M^T [B,128,128] (block-diagonal) and beta [B,128,1]."""
    N = float(HW)
    t = t.reshape(HEADS)
    M = np.zeros((B, C, C), dtype=np.float64)
    beta = np.zeros((B, C), dtype=np.float64)
    for b in range(B):
        for n in range(NCH):
            sl = slice(n * D, (n + 1) * D)
            Gb = G[b][sl, sl]
            dG = np.diag(Gb)
            Sb = S[b][sl]
            Mn = np.zeros((D, D), dtype=np.float64)
            bn = np.zeros(D, dtype=np.float64)
            for e in range(E):
                h = e * NCH + n
                A = w_qkv[sl, e]
                Bv = b_qkv[sl, e]
                Cv = w_qkv[sl, E + e]
                Dv = b_qkv[sl, E + e]
                Vv = w_qkv[sl, 2 * E + e]
                Uv = b_qkv[sl, 2 * E + e]
                qk = ((A[:, None] * Cv[None, :]) * Gb
                      + (A * Sb)[:, None] * Dv[None, :]
                      + Bv[:, None] * (Cv * Sb)[None, :]
                      + N * (Bv[:, None] * Dv[None, :]))
                nq = np.sqrt(np.maximum(A * A * dG + 2 * A * Bv * Sb + Bv * Bv * N, 0.0))
                nk = np.sqrt(np.maximum(Cv * Cv * dG + 2 * Cv * Dv * Sb + Dv * Dv * N, 0.0))
                L = t[h] * qk / np.maximum(nq, EPS)[:, None] / np.maximum(nk, EPS)[None, :]
                P = _softmax(L, axis=-1)
                Mn += w_fus[sl, e][:, None] * (P * Vv[None, :])
                bn += w_fus[sl, e] * (P @ Uv)
            bn += b_fus[sl]
            M[b][sl, sl] = Mn
            beta[b][sl] = bn
    mtr = np.ascontiguousarray(M.transpose(0, 2, 1)).astype(np.float32)
    return mtr, beta.astype(np.float32).reshape(B, C, 1)


def kernel(x, w_qkv, b_qkv, w_fus, b_fus, t, _profile=None):
    x = np.asarray(x, dtype=np.float32)
    w_qkv = np.asarray(w_qkv, dtype=np.float64)
    b_qkv = np.asarray(b_qkv, dtype=np.float64)
    w_fus = np.asarray(w_fus, dtype=np.float64)
    b_fus = np.asarray(b_fus, dtype=np.float64)
    t = np.asarray(t, dtype=np.float64)

    mode = (_profile or {}).get("p1mode", "pet")
    key1 = f"p1-{mode}"
    if key1 not in _cache:
        _cache[key1] = _build_phase1(mode)
    if "p2" not in _cache:
        _cache["p2"] = _build_phase2()

    import ml_dtypes
    xf = x.reshape(B, C, HW)
    shards = [np.ascontiguousarray(xf[:, :, i * SH:(i + 1) * SH])
              for i in range(NCORES)]

    kw = {}
    if _profile and _profile.get("trace"):
        kw["trace"] = True
    idd = np.eye(C, dtype=ml_dtypes.bfloat16)
    res1 = bass_utils.run_bass_kernel_spmd(
        _cache[key1], [{"x": s, "idd": idd} for s in shards],
        core_ids=list(range(NCORES)), **kw)
    gs = np.sum([r["gs"].astype(np.float64) for r in res1.results], axis=0)
    G = gs[:, :, 0:128]
    S = gs[:, :, 128]

    mtr, beta = _host_mbeta(G, S, w_qkv, b_qkv, w_fus, b_fus, t)
    res2 = bass_utils.run_bass_kernel_spmd(
        _cache["p2"],
        [{"x": s, "mt": mtr, "beta": beta} for s in shards],
        core_ids=list(range(NCORES)), **kw)
    out = np.concatenate([r["out"] for r in res2.results], axis=2)
    if _profile is not None:
        _profile["results"] = (res1, res2)
    return out.reshape(B, C, H, W)

